# revision 52
# baseline (speedup 1.0000x reference)
"""AttentionBlock (GroupNorm32 + 8-head global self-attention + proj + residual)
on 8 TRN2 NeuronCores, data-parallel over batch (B=8 -> 1 image per core).

Per-core layout ([C=512, N=1024] slice, channels on partitions):
  Startup: x shipped twice (bf16 half-tile DMAs over sync/scalar/gpsimd for
  the GN critical path, f32 late for the residual); qkv weights DRAM-permuted
  into consumption order; ~12 zero matmuls warm the PE HAM clock gate during
  the x DMA; a dummy Sqrt pre-loads the Sqrt/Exp ACT table set.
  GroupNorm: cts 0-2 bn_stats on DVE, ct3 on ACT (Copy/Square accum_out row
  sums); single selector/expander matmuls; affine applied in n-halves.
  Attention: S^T per (m-tile, head-parity) into a 3-tile psS rotation; exp
  split ACT (Exp) / DVE (Schraudolph int16 bit-trick).  PV per (head, nt)
  chain accumulates [66,512] with a ones-row denominator.  Normalization:
  chains closing early in a pair bounce their reciprocal through DRAM into a
  64-row broadcast (latency-tolerant, engine-cheap); late-closing chains use
  a self-tile PE broadcast (ones_b matmul into rows 64:128 of the chain's
  own psV tile).  Pair 3 emits S^T hh-major so its hh0 PV chains unblock
  before the last exps land.  proj: per-(ot, nt) half-chains land in psS
  tiles as S^T(3) releases them (+psVa/psVb for ot3); evac = residual add
  (proj bias host-folded into x) split across DVE and ACT+gpsimd, with
  per-half output DMAs on sync/scalar.
"""
import math

import numpy as np

C = 512
NH = 8
D = 64
N = 1024
GROUPS = 32
GS = C // GROUPS  # 16 channels per group
EPS = 1e-5
B = 8
NT = N // 512     # 2 n-tiles of 512
CT = C // 128     # 4 channel tiles
MT = N // 128     # 8 m-tiles (sequence on partitions)

TRACE = False     # test.py flips this for profiling runs

_cache = {}


def _build(with_bias):
    import concourse.bass as bass
    import concourse.bacc as bacc
    import concourse.tile as tile
    import concourse.mybir as mybir

    F32 = mybir.dt.float32
    F32R = mybir.dt.float32r
    BF16 = mybir.dt.bfloat16
    I16 = mybir.dt.int16
    AF = mybir.ActivationFunctionType
    ALU = mybir.AluOpType
    nc = bacc.Bacc("TRN2", target_bir_lowering=False, debug=False,
                   enable_asserts=False, num_devices=1)

    x_d = nc.dram_tensor("x", [C, N], F32, kind="ExternalInput").ap()
    xbf_d = nc.dram_tensor("x_bf", [C, N], BF16, kind="ExternalInput").ap()
    # qkv weights pre-permuted on host into consumption order:
    # [q0 | k0 | v(all 512) | q1 | k1 | q2 | k2 | q3 | k3] (128-col blocks)
    qkv_wT_d = nc.dram_tensor("qkv_blk", [C, 3 * C], BF16, kind="ExternalInput").ap()
    proj_wT_d = nc.dram_tensor("proj_wT", [C, C], BF16, kind="ExternalInput").ap()
    qk_bias_d = nc.dram_tensor("qk_bias", [2 * C, 1], F32, kind="ExternalInput").ap()
    gn_w_d = nc.dram_tensor("gn_w", [C, 1], F32, kind="ExternalInput").ap()
    gn_b_d = nc.dram_tensor("gn_b", [C, 1], F32, kind="ExternalInput").ap()
    sel_d = nc.dram_tensor("sel", [128, 8], F32R, kind="ExternalInput").ap()
    expander_d = nc.dram_tensor("expander", [8, 128], F32R, kind="ExternalInput").ap()
    rs2_dram = nc.dram_tensor("rs2_scratch", [NH, N], F32, kind="Internal").ap()
    out_d = nc.dram_tensor("out", [C, N], F32, kind="ExternalOutput").ap()

    x_r = x_d.rearrange("(t p) n -> p t n", p=128)
    xbf_r = xbf_d.rearrange("(t p) n -> p t n", p=128)
    qkv_r = qkv_wT_d.rearrange("(t p) o -> p t o", p=128)
    proj_r = proj_wT_d.rearrange("(t p) o -> p t o", p=128)
    out_r = out_d.rearrange("(t p) n -> p t n", p=128)

    scale = float(D) ** -0.5
    # Schraudolph exp-as-bf16: bf16_bits(exp(scale*s)) ~= round(A*s + B)
    SCH_A = (2.0 ** 23) / math.log(2.0) / 65536.0 * scale
    SCH_B = (127.0 * 2 ** 23 - 368000.0) / 65536.0
    # which S^T emission positions run exp on DVE instead of ACT, pairs 1-3
    EXP_DVE = {1, 4, 7, 10, 13}

    with tile.TileContext(nc) as tc:
        with tc.tile_pool(name="const", bufs=1) as const, \
             tc.tile_pool(name="big", bufs=1) as big, \
             tc.tile_pool(name="pT_pool", bufs=4) as pT_pool, \
             tc.tile_pool(name="small", bufs=2) as small, \
             tc.tile_pool(name="norm", bufs=4) as norm, \
             tc.tile_pool(name="psSa_p", bufs=1, space="PSUM") as psSa_p, \
             tc.tile_pool(name="psSb_p", bufs=1, space="PSUM") as psSb_p, \
             tc.tile_pool(name="psSc_p", bufs=1, space="PSUM") as psSc_p, \
             tc.tile_pool(name="psVa_p", bufs=1, space="PSUM") as psVa_p, \
             tc.tile_pool(name="psVb_p", bufs=1, space="PSUM") as psVb_p:

            # ---- PSUM: 5 fixed tiles (8 banks). Separate tiles because the
            # dependency tracker serializes at tile granularity; S^T/exp uses
            # a 3-buffer rotation so the engine throughput (not the serial
            # st->exp chain) bounds the cadence.
            psSa = psSa_p.tile([128, 1024], F32)  # S^T rot 0, q0, proj 0/3
            psSb = psSb_p.tile([128, 1024], F32)  # S^T rot 1, k0, proj 1
            psSc = psSc_p.tile([128, 1024], F32)  # S^T rot 2, GN, qk rest, proj 2
            psVa = psVa_p.tile([128, 512], F32)   # v even, PV chains 0,2
            psVb = psVb_p.tile([128, 512], F32)   # v odd, PV chains 1,3

            # ---- constants / weights (gpsimd queue) ----
            sel = const.tile([128, 8], F32R)
            expander = const.tile([8, 128], F32R)
            gn_w = const.tile([128, CT, 1], F32)
            gn_b = const.tile([128, CT, 1], F32)
            qkv_wT = const.tile([128, CT, 3 * C], BF16)
            proj_wT = const.tile([128, CT, C], BF16)
            eps_t = const.tile([8, 1], F32)
            ones_b = const.tile([1, 64], BF16)
            qk_bias = const.tile([128, 2 * CT, 1], F32)
            warm = const.tile([128, 640], BF16)

            # ---- input x (bf16, critical path): 8 half-tile DMAs over 3
            # queues, issued before anything else, so bn_stats can start on
            # the first 512-col half while later halves are still in flight
            xb_sb = big.tile([128, CT, N], BF16)
            for ci, h in ((0, 0), (0, 1), (2, 0), (2, 1)):
                nc.sync.dma_start(out=xb_sb[:, ci, 512 * h:512 * (h + 1)],
                                  in_=xbf_r[:, ci, 512 * h:512 * (h + 1)])
            for h in range(2):
                nc.scalar.dma_start(out=xb_sb[:, 1, 512 * h:512 * (h + 1)],
                                    in_=xbf_r[:, 1, 512 * h:512 * (h + 1)])

            # ---- PE warm-up: ~10 dummy matmuls on zeros keep the HAM clock
            # gate from throttling the first real matmul stream (PE would
            # otherwise sit idle >3.4us during the x DMA + GN stats). ----
            nc.gpsimd.memset(warm, 0.0)
            for h in range(2):
                nc.gpsimd.dma_start(out=xb_sb[:, 3, 512 * h:512 * (h + 1)],
                                    in_=xbf_r[:, 3, 512 * h:512 * (h + 1)])

            def warm_mm(n):
                for i in range(n):
                    nc.tensor.matmul((psVa if i % 2 == 0 else psVb)[:, 0:512],
                                     warm[:, 0:128], warm[:, 0:512],
                                     start=True, stop=True)

            warm_mm(12)

            # ---- weights (gpsimd queue) in consumption order ----
            # blk col ranges in qkv_blk -> original qkv_wT col ranges
            BLK_MAP = [(0, 0), (128, 512), (256, 1024, 512), (768, 128),
                       (896, 640), (1024, 256), (1152, 768), (1280, 384),
                       (1408, 896)]

            def qkv_load(idx):
                e = BLK_MAP[idx]
                w = e[2] if len(e) == 3 else 128
                nc.gpsimd.dma_start(out=qkv_wT[:, :, e[1]:e[1] + w],
                                    in_=qkv_r[:, :, e[0]:e[0] + w])

            qkv_load(0)                      # q0
            nc.scalar.dma_start(out=sel, in_=sel_d)
            nc.scalar.dma_start(out=expander, in_=expander_d)
            nc.scalar.dma_start(out=gn_w, in_=gn_w_d.rearrange("(t p) o -> p t o", p=128))
            nc.scalar.dma_start(out=gn_b, in_=gn_b_d.rearrange("(t p) o -> p t o", p=128))
            if with_bias:
                nc.scalar.dma_start(out=qk_bias,
                                    in_=qk_bias_d.rearrange("(t p) o -> p t o", p=128))
            qkv_load(1)                      # k0
            qkv_load(2)                      # v (all 512 cols)
            for idx in range(3, 9):          # q1,k1,q2,k2,q3,k3
                qkv_load(idx)
            nc.gpsimd.dma_start(out=proj_wT, in_=proj_r)
            nc.vector.memset(eps_t, EPS)
            nc.vector.memset(ones_b, 1.0)
            # dummy sqrt: forces the Sqrt/Exp ACT table set to load NOW
            # (ACT idle) instead of mid GN chain
            tbl_poke = const.tile([8, 1], F32)
            nc.scalar.activation(out=tbl_poke, in_=eps_t, func=AF.Sqrt)

            # f32 x for the residual, loaded behind the weights (gpsimd)
            x_sb = big.tile([128, CT, N], F32)
            nc.gpsimd.dma_start(out=x_sb, in_=x_r)

            # ---- GroupNorm: cts 0-2 stats on DVE, ct3 on ACT (accum_out =
            # free-dim sums of x and x^2); single sel/expander matmuls ----
            hn = big.tile([128, CT, N], BF16)
            out_sb = big.tile([128, CT, N], F32)
            mv_all = norm.tile([128, CT, 2], F32, bufs=1)
            sums3 = norm.tile([128, 2], F32, bufs=1)
            for ci in range(3):
                bstats = norm.tile([128, 2, 6], F32, tag="bst")
                xv = xb_sb[:, ci, :].rearrange("p (s n) -> p s n", s=2)
                for s in range(2):
                    nc.vector.bn_stats(out=bstats[:, s, :], in_=xv[:, s, :])
                nc.vector.bn_aggr(out=mv_all[:, ci, :], in_=bstats)
            # ct3 on ACT; scratch outputs land in tiles rewritten later
            nc.scalar.activation(out=out_sb[:, 0, :], in_=xb_sb[:, 3, :],
                                 func=AF.Copy, accum_out=sums3[:, 0:1])
            nc.scalar.activation(out=hn[:, 3, :], in_=xb_sb[:, 3, :],
                                 func=AF.Square, accum_out=sums3[:, 1:2])
            # srhs: col0 = mean_c, col1 = E[x^2]
            srhs = norm.tile([128, CT, 2], F32R, bufs=1)
            nc.vector.tensor_copy(out=srhs[:, 0:3, 0], in_=mv_all[:, 0:3, 0])
            nc.vector.tensor_tensor(out=srhs[:, 0:3, 1], in0=mv_all[:, 0:3, 0],
                                    in1=mv_all[:, 0:3, 0], op=ALU.mult)
            nc.vector.tensor_tensor(out=srhs[:, 0:3, 1], in0=srhs[:, 0:3, 1],
                                    in1=mv_all[:, 0:3, 1], op=ALU.add)
            nc.vector.tensor_scalar(out=srhs[:, 3, :], in0=sums3,
                                    scalar1=1.0 / N, scalar2=None, op0=ALU.mult)
            gp = psSc[0:8, 512:512 + 2 * CT]
            nc.tensor.matmul(gp, sel[:], srhs[:], start=True, stop=True)
            warm_mm(4)
            gms = norm.tile([8, CT, 2], F32, bufs=1)
            nc.vector.tensor_copy(out=gms,
                                  in_=gp.rearrange("p (c s) -> p c s", s=2))
            gvar = norm.tile([8, CT], F32, bufs=1)
            grp2 = norm.tile([8, CT, 2], F32R, bufs=1)
            nc.vector.tensor_tensor(out=gvar, in0=gms[:, :, 0], in1=gms[:, :, 0],
                                    op=ALU.mult)
            nc.vector.tensor_tensor(out=gvar, in0=gms[:, :, 1], in1=gvar,
                                    op=ALU.subtract)
            nc.scalar.activation(out=gvar, in_=gvar, func=AF.Sqrt, bias=eps_t,
                                 scale=1.0)
            nc.vector.reciprocal(out=gvar, in_=gvar)
            nc.vector.tensor_copy(out=grp2[:, :, 0:1], in_=gms[:, :, 0:1])
            nc.vector.tensor_copy(out=grp2[:, :, 1],
                                  in_=gvar)
            ep = psSc[:, 576:576 + 2 * CT]
            nc.tensor.matmul(ep, expander[:], grp2[:], start=True, stop=True)
            warm_mm(3)
            eab = norm.tile([128, CT, 2], F32, bufs=1)
            nc.vector.tensor_copy(out=eab,
                                  in_=ep.rearrange("p (c s) -> p c s", s=2))
            A_all = norm.tile([128, CT], F32, bufs=1)
            B_all = norm.tile([128, CT], F32, bufs=1)
            nc.vector.tensor_tensor(out=A_all, in0=eab[:, :, 1],
                                    in1=gn_w[:, :, 0], op=ALU.mult)
            nc.vector.tensor_tensor(out=B_all, in0=eab[:, :, 0], in1=A_all,
                                    op=ALU.mult)
            nc.vector.tensor_tensor(out=B_all, in0=gn_b[:, :, 0], in1=B_all,
                                    op=ALU.subtract)
            # affine split by n-halves so the qk chains (nt0 first) unblock
            # after 4 half-ops instead of 4 full ones
            for ntv in range(NT):
                for ci in range(CT):
                    nc.vector.tensor_scalar(
                        out=hn[:, ci, 512 * ntv:512 * (ntv + 1)],
                        in0=xb_sb[:, ci, 512 * ntv:512 * (ntv + 1)],
                        scalar1=A_all[:, ci:ci + 1],
                        scalar2=B_all[:, ci:ci + 1],
                        op0=ALU.mult, op1=ALU.add)

            # ---- data tiles for attention ----
            q_sb = big.tile([128, CT, N], BF16)
            k_sb = big.tile([128, CT, N], BF16)
            vT = big.tile([128, MT, NH, D + 2], BF16)
            oT = big.tile([128, CT, N], BF16)
            nc.vector.memset(vT[:, :, :, D:D + 1], 1.0)
            nc.vector.memset(vT[:, :, :, D + 1:D + 2], 0.0)

            # ---- emission helpers ----
            def qk_tile(i, on_act, bases):
                """QKV tile i (0-3 = q ct, 4-7 = k ct). nt-outer: the nt1
                chain's first MM is FIFO-gated behind nt0's kt3 (needs hn3),
                so psSc's GN half is only written once GN is done."""
                dest = q_sb if i < CT else k_sb
                ci = i % CT
                base = bases[0]
                for nt in range(NT):
                    for kt in range(CT):
                        nc.tensor.matmul(
                            base[:, 512 * nt:512 * (nt + 1)],
                            qkv_wT[:, kt, 128 * i:128 * (i + 1)],
                            hn[:, kt, 512 * nt:512 * (nt + 1)],
                            start=(kt == 0), stop=(kt == CT - 1))
                if with_bias:
                    nc.vector.tensor_scalar(out=dest[:, ci, :],
                                            in0=base[:, 0:1024],
                                            scalar1=qk_bias[:, i, :],
                                            scalar2=None, op0=ALU.add)
                elif on_act:
                    nc.scalar.activation(out=dest[:, ci, :],
                                         in_=base[:, 0:1024], func=AF.Copy)
                else:
                    nc.vector.tensor_copy(out=dest[:, ci, :],
                                          in_=base[:, 0:1024])

            def v_tile(mt):
                """v for n-block mt, evac to vT (head-interleaved)."""
                base = psVa if mt % 2 == 0 else psVb
                for kt in range(CT):
                    nc.tensor.matmul(base[:, 0:512],
                                     hn[:, kt, 128 * mt:128 * (mt + 1)],
                                     qkv_wT[:, kt, 2 * C:3 * C],
                                     start=(kt == 0), stop=(kt == CT - 1))
                nc.vector.tensor_copy(
                    out=vT[:, mt, :, 0:D],
                    in_=base[:, 0:512].rearrange("p (h d) -> p h d", h=NH))

            pT_tiles = {}

            def st_half(t, g, on_dve, half):
                """S^T for head pair t, group g = 2*mt + hh, into psS tile
                `half`; exp on ACT or DVE-Schraudolph."""
                mt, hh = g // 2, g % 2
                qp = hh * 64
                for nt in range(NT):
                    nc.tensor.matmul(
                        half[:, 512 * nt:512 * (nt + 1)],
                        k_sb[qp:qp + 64, t, 128 * mt:128 * (mt + 1)],
                        q_sb[qp:qp + 64, t, 512 * nt:512 * (nt + 1)],
                        start=True, stop=True)
                pTt = pT_tiles[t]
                if on_dve:
                    nc.vector.tensor_scalar(
                        out=pTt.bitcast(I16)[:, hh, mt, :], in0=half,
                        scalar1=SCH_A, scalar2=SCH_B,
                        op0=ALU.mult, op1=ALU.add)
                else:
                    nc.scalar.activation(out=pTt[:, hh, mt, :], in_=half,
                                         func=AF.Exp, scale=scale)

            # Chains 2,3 of each pair close EARLY (position 2/6) and use the
            # DRAM-bounce reciprocal broadcast (long latency, no PE cost).
            # Chains 0,1 close LATE (position 11/15) and use the low-latency
            # self-tile PE broadcast (rows 64:128 of their own psV tile).
            # Pair 3 (the tail) is all self-tile.
            def pv_tile(t, chain):
                if t == 3:
                    # hh-major: chains 0,1 (nt0) in psVa; 2,3 (nt1) in psVb
                    return psVa if chain in (0, 1) else psVb
                return psVa if chain % 2 == 0 else psVb

            def pv_chunk(t, chain, part):
                """4 MMs of PV chain (0=h_ev/nt0, 1=h_od/nt0, 2=h_ev/nt1,
                3=h_od/nt1), part 0/1 = m-tiles 0-3 / 4-7."""
                hh = chain % 2
                nt = chain // 2
                h = 2 * t + hh
                tile_ = pv_tile(t, chain)
                slot = tile_[0:D + 2, 0:512]
                pTt = pT_tiles[t]
                for mt in range(4 * part, 4 * part + 4):
                    nc.tensor.matmul(slot,
                                     vT[:, mt, h, :],
                                     pTt[:, hh, mt, 512 * nt:512 * (nt + 1)],
                                     start=(mt == 0), stop=(mt == MT - 1))
                if part == 1:
                    if t < 3 and chain in (2, 3):
                        # denominator row -> SBUF, approx reciprocal, then
                        # DRAM-bounce broadcast of the reciprocal to 64 rows
                        rs = small.tile([1, 512], F32, tag="rs", bufs=4,
                                        name=f"rs_{h}_{nt}")
                        rs2 = small.tile([1, 512], F32, tag="rs2", bufs=4,
                                         name=f"rs2_{h}_{nt}")
                        nc.vector.tensor_copy(out=rs,
                                              in_=tile_[D:D + 1, 0:512])
                        nc.vector.reciprocal_approx_fast(out=rs2, in_=rs)
                        nc.sync.dma_start(
                            out=rs2_dram[h:h + 1, 512 * nt:512 * (nt + 1)],
                            in_=rs2)
                        bc_load(t, chain)
                    else:
                        denom_self(t, chain)

            bc_tiles = {}

            def bc_load(t, chain):
                """Broadcast 1/denom to 64 partitions via DRAM-bounce DMA."""
                hh, nt = chain % 2, chain // 2
                h = 2 * t + hh
                qp = hh * 64
                key = (t, nt)
                if key not in bc_tiles:
                    bc_tiles[key] = small.tile([128, 512], F32, tag=f"bc{nt}",
                                               bufs=2, name=f"bc_{t}_{nt}")
                bc = bc_tiles[key]
                srcap = rs2_dram[h:h + 1, 512 * nt:512 * (nt + 1)]
                nc.gpsimd.dma_start(out=bc[qp:qp + 64, :],
                                    in_=bass.AP(tensor=srcap.tensor,
                                                offset=srcap.offset,
                                                ap=[[0, 64]] + list(srcap.ap[1:])))

            def pv_evac(t, chain):
                """Fused evacuate+normalize: oT = psum_o * (1/denom)."""
                hh, nt = chain % 2, chain // 2
                qp = hh * 64
                tile_ = pv_tile(t, chain)
                bc = bc_tiles[(t, nt)]
                nc.vector.tensor_tensor(
                    out=oT[qp:qp + 64, t, 512 * nt:512 * (nt + 1)],
                    in0=tile_[0:D, 0:512], in1=bc[qp:qp + 64, :], op=ALU.mult)

            def denom_self(t, chain):
                """Recip of the psum denominator row, bf16-cast on ACT, then
                PE-broadcast into the SAME psV tile's rows 64:128 (no
                cross-tile lattice, no DRAM bounce latency)."""
                tile_ = pv_tile(t, chain)
                rs = small.tile([1, 512], F32, tag="rs", bufs=4,
                                name=f"rst_{t}_{chain}")
                rs2 = small.tile([1, 512], F32, tag="rs2", bufs=4,
                                 name=f"rs2t_{t}_{chain}")
                rs2b = small.tile([1, 512], BF16, tag="rs2b", bufs=4,
                                  name=f"rs2b_{t}_{chain}")
                # rs -> recip -> bf16 cast all back-to-back on DVE: no ACT
                # queue hop on the chain's critical path (ACT tail queue has
                # head-of-line blocking with 1-7us waits)
                nc.vector.tensor_copy(out=rs, in_=tile_[D:D + 1, 0:512])
                nc.vector.reciprocal_approx_fast(out=rs2, in_=rs)
                nc.vector.tensor_copy(out=rs2b, in_=rs2)
                nc.tensor.matmul(tile_[64:128, 0:512], ones_b, rs2b,
                                 start=True, stop=True)

            def pv_evac_self(t, chain):
                hh, nt = chain % 2, chain // 2
                qp = hh * 64
                tile_ = pv_tile(t, chain)
                # stage the broadcast rows to SBUF (ACT when idle in the
                # tail, DVE mid-kernel where ACT is exp-bound), multiply on
                # DVE.  (Both-operands-in-PSUM TT crashes the backend.)
                bcs = small.tile([64, 512], F32, tag="bcs", bufs=2,
                                 name=f"bcs_{t}_{chain}")
                if t == 3:
                    nc.scalar.activation(out=bcs, in_=tile_[64:128, 0:512],
                                         func=AF.Copy)
                else:
                    nc.vector.tensor_copy(out=bcs, in_=tile_[64:128, 0:512])
                nc.vector.tensor_tensor(
                    out=oT[qp:qp + 64, t, 512 * nt:512 * (nt + 1)],
                    in0=tile_[0:D, 0:512], in1=bcs, op=ALU.mult)

            def proj_part(ot, nt, kts, base, bc0, first=False, last=False):
                for kt in kts:
                    nc.tensor.matmul(base[:, bc0:bc0 + 512],
                                     proj_wT[:, kt, 128 * ot:128 * (ot + 1)],
                                     oT[:, kt, 512 * nt:512 * (nt + 1)],
                                     start=(first and kt == kts[0]),
                                     stop=(last and kt == kts[-1]))

            def proj_out(ot, nt, base, bc0, eng):
                """Evacuate one proj half + residual (proj bias folded into
                x on host) and DMA it out. eng 0 = DVE, 1 = ACT + gpsimd."""
                ocols = slice(512 * nt, 512 * (nt + 1))
                if eng == 0:
                    nc.vector.tensor_tensor(out=out_sb[:, ot, ocols],
                                            in0=base[:, bc0:bc0 + 512],
                                            in1=x_sb[:, ot, ocols], op=ALU.add)
                else:
                    nc.scalar.activation(out=out_sb[:, ot, ocols],
                                         in_=base[:, bc0:bc0 + 512],
                                         func=AF.Copy)
                    nc.gpsimd.tensor_tensor(out=out_sb[:, ot, ocols],
                                            in0=out_sb[:, ot, ocols],
                                            in1=x_sb[:, ot, ocols],
                                            op=ALU.add)
                q = nc.sync if (ot + nt) % 2 == 0 else nc.scalar
                q.dma_start(out=out_r[:, ot, 512 * nt:512 * (nt + 1)],
                            in_=out_sb[:, ot, ocols])

            def alloc_pT(t):
                pT_tiles[t] = pT_pool.tile([128, 2, MT, N], BF16, tag="pT", bufs=2,
                                           name=f"pT_{t}")

            # ---- pipeline emission ----
            # q0, k0 first (psSa/psSb) so pair-0 S^T can start early
            qk_tile(0, on_act=True, bases=[psSa])
            qk_tile(4, on_act=True, bases=[psSb])

            # pair 0: S^T+exp (all ACT; PE-bound anyway) + v tiles (psV) +
            # remaining qk tiles (psX)
            alloc_pT(0)
            rest = [1, 5, 2, 6, 3, 7]        # q1,k1,q2,k2,q3,k3
            p0rot = [psSa, psSb]
            for g in range(16):
                if g % 2 == 0:
                    v_tile(g // 2)
                elif g < 13:
                    # q1/k1 evacs ride ACT (DVE is pair-0's busiest engine)
                    qk_tile(rest[g // 2], on_act=(g < 5), bases=[psSc])
                st_half(0, g, on_dve=False, half=p0rot[g % 2])

            # pairs 1..3: PV(t-1) + S^T(t) + exp (ACT/DVE split) + stage_b.
            # pair 3 emits S^T hh-major so PV(3)'s hh0 chains unblock before
            # the last exps land.
            for t in range(1, 4):
                alloc_pT(t)
                pv = t - 1
                chunk_sched = {0: (2, 0), 2: (2, 1), 4: (3, 0), 6: (3, 1),
                               9: (0, 0), 11: (0, 1), 13: (1, 0), 15: (1, 1)}
                order = ([0, 2, 4, 6, 8, 10, 12, 14, 1, 3, 5, 7, 9, 11, 13, 15]
                         if t == 3 else list(range(16)))
                rot = [psSa, psSb, psSc]
                for idx in range(16):
                    if idx == 1 and pv >= 1:
                        pv_evac_self(pv - 1, 1)
                    cs = chunk_sched.get(idx)
                    if cs is not None:
                        pv_chunk(pv, cs[0], cs[1])
                    st_half(t, order[idx], on_dve=(idx in EXP_DVE),
                            half=rot[idx % 3])
                    if idx == 8:
                        pv_evac(pv, 2)
                    elif idx == 12:
                        pv_evac(pv, 3)
                    elif idx == 13:
                        pv_evac_self(pv, 0)
                    if t == 3:
                        # merge the tail into the loop: PV(3,0) + the first
                        # proj chains start as soon as their tiles free.
                        # ot0-nt0 stops at kt1: its kt2 rows 64:128 need
                        # chain (2,1)'s evac, which happens at tail start.
                        if idx == 14:
                            pv_chunk(3, 0, 0)
                            pv_chunk(3, 0, 1)
                        elif idx == 15:
                            proj_part(0, 0, [0, 1], psSb, 0, first=True)
                            proj_part(0, 1, [0, 1, 2], psSb, 512, first=True)
                del pT_tiles[t - 1]

            # ---- tail: PV(3) hh0 chains first, per-(ot,nt) proj chains with
            # early half-tile evac + DMA; denominators via self-tile PE
            # broadcast (rows 64:128 of the chain's own psV tile). ----
            pv_evac_self(2, 1)                    # frees psVb
            pv_chunk(3, 2, 0)
            pv_chunk(3, 2, 1)                     # psVb chain2 (hh0 nt1)
            proj_part(0, 0, [2], psSb, 0)         # kt2 after evac(2,1)
            proj_part(1, 0, [0, 1, 2], psSc, 0, first=True)
            pv_evac_self(3, 0)                    # oT[0:64, 3, 0:512]
            proj_part(1, 1, [0, 1, 2], psSc, 512, first=True)
            pv_evac_self(3, 2)                    # oT[0:64, 3, 512:1024]
            pv_chunk(3, 1, 0)
            pv_chunk(3, 1, 1)                     # psVa chain1 (hh1 nt0)
            proj_part(2, 0, [0, 1, 2], psSa, 0, first=True)
            pv_evac_self(3, 1)                    # oT[64:128, 3, 0:512]
            proj_part(0, 0, [3], psSb, 0, last=True)
            proj_out(0, 0, psSb, 0, eng=1)
            pv_chunk(3, 3, 0)
            pv_chunk(3, 3, 1)                     # psVb chain3 (hh1 nt1)
            proj_part(2, 1, [0, 1, 2], psSa, 512, first=True)
            proj_part(1, 0, [3], psSc, 0, last=True)
            proj_out(1, 0, psSc, 0, eng=0)
            proj_part(3, 0, [0, 1, 2, 3], psVa, 0, first=True, last=True)
            proj_out(3, 0, psVa, 0, eng=1)
            proj_part(2, 0, [3], psSa, 0, last=True)
            proj_out(2, 0, psSa, 0, eng=0)
            pv_evac_self(3, 3)                    # oT[64:128, 3, 512:1024]
            proj_part(0, 1, [3], psSb, 512, last=True)
            proj_out(0, 1, psSb, 512, eng=1)
            proj_part(3, 1, [0, 1, 2, 3], psVb, 0, first=True, last=True)
            proj_part(1, 1, [3], psSc, 512, last=True)
            proj_out(1, 1, psSc, 512, eng=0)
            proj_out(3, 1, psVb, 0, eng=0)
            proj_part(2, 1, [3], psSa, 512, last=True)
            proj_out(2, 1, psSa, 512, eng=0)

    nc.compile()
    return nc


def _host_prep(x, gn_w, gn_b, qkv_w, qkv_b, proj_w, proj_b):
    xf = np.ascontiguousarray(x.reshape(B, C, N), dtype=np.float32)
    import ml_dtypes
    qkv_wT = np.ascontiguousarray(qkv_w.T).astype(ml_dtypes.bfloat16)
    # permute columns into consumption order (see BLK_MAP in _build)
    qkv_blk = np.ascontiguousarray(np.concatenate(
        [qkv_wT[:, 0:128], qkv_wT[:, 512:640], qkv_wT[:, 1024:1536],
         qkv_wT[:, 128:256], qkv_wT[:, 640:768], qkv_wT[:, 256:384],
         qkv_wT[:, 768:896], qkv_wT[:, 384:512], qkv_wT[:, 896:1024]],
        axis=1))
    proj_wT = np.ascontiguousarray(proj_w.T).astype(ml_dtypes.bfloat16)
    # fold the proj bias (incl. v-bias pushed through proj_w) into the
    # residual copy of x; the bf16 GN input stays un-folded
    proj_be = (proj_b + proj_w @ qkv_b[2 * C:]).astype(np.float32)
    qk_bias = np.ascontiguousarray(qkv_b[:2 * C], dtype=np.float32).reshape(2 * C, 1)
    cid = np.arange(128)
    sel = ((cid[:, None] // GS == np.arange(8)[None, :]) / GS).astype(np.float32)
    expander = np.ascontiguousarray(
        (cid[:, None] // GS == np.arange(8)[None, :]).T.astype(np.float32))
    shared = {
        "qkv_blk": qkv_blk, "proj_wT": proj_wT, "qk_bias": qk_bias,
        "gn_w": np.asarray(gn_w, np.float32).reshape(C, 1),
        "gn_b": np.asarray(gn_b, np.float32).reshape(C, 1),
        "sel": sel, "expander": expander,
    }
    return [{**shared,
             "x": np.ascontiguousarray(xf[i] + proj_be[:, None]),
             "x_bf": xf[i].astype(ml_dtypes.bfloat16)} for i in range(B)]


def kernel(x, gn_w, gn_b, qkv_w, qkv_b, proj_w, proj_b):
    from concourse import bass_utils
    in_maps = _host_prep(np.asarray(x), np.asarray(gn_w), np.asarray(gn_b),
                         np.asarray(qkv_w), np.asarray(qkv_b),
                         np.asarray(proj_w), np.asarray(proj_b))
    with_bias = bool(np.any(np.asarray(qkv_b)[:2 * C] != 0.0))
    key = ("nc", with_bias)
    if key not in _cache:
        _cache[key] = _build(with_bias)
    res = bass_utils.run_bass_kernel_spmd(_cache[key], in_maps,
                                          core_ids=list(range(B)), trace=TRACE)
    _cache["last_result"] = res
    out = np.stack([res.results[i]["out"] for i in range(B)])
    return out.reshape(B, C, 32, 32).astype(np.float32)



# revision 54
# speedup vs baseline: 1.2313x; 1.2313x over previous
"""AttentionBlock (GroupNorm32 + 8-head global self-attention + proj + residual)
on 8 TRN2 NeuronCores, data-parallel over batch (B=8 -> 1 image per core).

Per-core layout ([C=512, N=1024] slice, channels on partitions):
  Startup: x shipped twice (bf16 half-tile DMAs over sync/scalar/gpsimd for
  the GN critical path, f32 late for the residual); qkv weights DRAM-permuted
  into consumption order; ~12 zero matmuls warm the PE HAM clock gate during
  the x DMA; a dummy Sqrt pre-loads the Sqrt/Exp ACT table set.
  GroupNorm: cts 0-2 bn_stats on DVE, ct3 on ACT (Copy/Square accum_out row
  sums); single selector/expander matmuls; affine applied in n-halves.
  Attention: S^T per (m-tile, head-parity) into a 3-tile psS rotation; exp
  split ACT (Exp) / DVE (Schraudolph int16 bit-trick).  PV per (head, nt)
  chain accumulates [66,512] with a ones-row denominator.  Normalization:
  chains closing early in a pair bounce their reciprocal through DRAM into a
  64-row broadcast (latency-tolerant, engine-cheap); late-closing chains use
  a self-tile PE broadcast (ones_b matmul into rows 64:128 of the chain's
  own psV tile).  Pair 3 emits S^T hh-major so its hh0 PV chains unblock
  before the last exps land.  proj: per-(ot, nt) half-chains land in psS
  tiles as S^T(3) releases them (+psVa/psVb for ot3); evac = residual add
  (proj bias host-folded into x) split across DVE and ACT+gpsimd, with
  per-half output DMAs on sync/scalar.
"""
import math

import numpy as np

C = 512
NH = 8
D = 64
N = 1024
GROUPS = 32
GS = C // GROUPS  # 16 channels per group
EPS = 1e-5
B = 8
NT = N // 512     # 2 n-tiles of 512
CT = C // 128     # 4 channel tiles
MT = N // 128     # 8 m-tiles (sequence on partitions)

TRACE = False     # test.py flips this for profiling runs

_cache = {}


def _build(with_bias):
    import concourse.bass as bass
    import concourse.bacc as bacc
    import concourse.tile as tile
    import concourse.mybir as mybir

    F32 = mybir.dt.float32
    F32R = mybir.dt.float32r
    BF16 = mybir.dt.bfloat16
    I16 = mybir.dt.int16
    AF = mybir.ActivationFunctionType
    ALU = mybir.AluOpType
    nc = bacc.Bacc("TRN2", target_bir_lowering=False, debug=False,
                   enable_asserts=False, num_devices=1)

    x_d = nc.dram_tensor("x", [C, N], F32, kind="ExternalInput").ap()
    xbf_d = nc.dram_tensor("x_bf", [C, N], BF16, kind="ExternalInput").ap()
    # qkv weights pre-permuted on host into consumption order:
    # [q0 | k0 | v(all 512) | q1 | k1 | q2 | k2 | q3 | k3] (128-col blocks)
    qkv_wT_d = nc.dram_tensor("qkv_blk", [C, 3 * C], BF16, kind="ExternalInput").ap()
    proj_wT_d = nc.dram_tensor("proj_wT", [C, C], BF16, kind="ExternalInput").ap()
    qk_bias_d = nc.dram_tensor("qk_bias", [2 * C, 1], F32, kind="ExternalInput").ap()
    gn_w_d = nc.dram_tensor("gn_w", [C, 1], F32, kind="ExternalInput").ap()
    gn_b_d = nc.dram_tensor("gn_b", [C, 1], F32, kind="ExternalInput").ap()
    sel_d = nc.dram_tensor("sel", [128, 8], F32R, kind="ExternalInput").ap()
    expander_d = nc.dram_tensor("expander", [8, 128], F32R, kind="ExternalInput").ap()
    rs2_dram = nc.dram_tensor("rs2_scratch", [NH, N], F32, kind="Internal").ap()
    out_d = nc.dram_tensor("out", [C, N], F32, kind="ExternalOutput").ap()

    x_r = x_d.rearrange("(t p) n -> p t n", p=128)
    xbf_r = xbf_d.rearrange("(t p) n -> p t n", p=128)
    qkv_r = qkv_wT_d.rearrange("(t p) o -> p t o", p=128)
    proj_r = proj_wT_d.rearrange("(t p) o -> p t o", p=128)
    out_r = out_d.rearrange("(t p) n -> p t n", p=128)

    scale = float(D) ** -0.5
    # Schraudolph exp-as-bf16: bf16_bits(exp(scale*s)) ~= round(A*s + B)
    SCH_A = (2.0 ** 23) / math.log(2.0) / 65536.0 * scale
    SCH_B = (127.0 * 2 ** 23 - 368000.0) / 65536.0
    # which S^T emission positions run exp on DVE instead of ACT, pairs 1-3
    EXP_DVE = {1, 4, 7, 10, 13}

    with tile.TileContext(nc) as tc:
        with tc.tile_pool(name="const", bufs=1) as const, \
             tc.tile_pool(name="big", bufs=1) as big, \
             tc.tile_pool(name="pT_pool", bufs=4) as pT_pool, \
             tc.tile_pool(name="small", bufs=2) as small, \
             tc.tile_pool(name="norm", bufs=4) as norm, \
             tc.tile_pool(name="psSa_p", bufs=1, space="PSUM") as psSa_p, \
             tc.tile_pool(name="psSb_p", bufs=1, space="PSUM") as psSb_p, \
             tc.tile_pool(name="psSc_p", bufs=1, space="PSUM") as psSc_p, \
             tc.tile_pool(name="psVa_p", bufs=1, space="PSUM") as psVa_p, \
             tc.tile_pool(name="psVb_p", bufs=1, space="PSUM") as psVb_p:

            # ---- PSUM: 5 fixed tiles (8 banks). Separate tiles because the
            # dependency tracker serializes at tile granularity; S^T/exp uses
            # a 3-buffer rotation so the engine throughput (not the serial
            # st->exp chain) bounds the cadence.
            psSa = psSa_p.tile([128, 1024], F32)  # S^T rot 0, q0, proj 0/3
            psSb = psSb_p.tile([128, 1024], F32)  # S^T rot 1, k0, proj 1
            psSc = psSc_p.tile([128, 1024], F32)  # S^T rot 2, GN, qk rest, proj 2
            psVa = psVa_p.tile([128, 512], F32)   # v even, PV chains 0,2
            psVb = psVb_p.tile([128, 512], F32)   # v odd, PV chains 1,3

            # ---- constants / weights (gpsimd queue) ----
            sel = const.tile([128, 8], F32R)
            expander = const.tile([8, 128], F32R)
            gn_w = const.tile([128, CT, 1], F32)
            gn_b = const.tile([128, CT, 1], F32)
            qkv_wT = const.tile([128, CT, 3 * C], BF16)
            proj_wT = const.tile([128, CT, C], BF16)
            eps_t = const.tile([8, 1], F32)
            ones_b = const.tile([1, 64], BF16)
            qk_bias = const.tile([128, 2 * CT, 1], F32)
            warm = const.tile([128, 640], BF16)

            # ---- input x (bf16, critical path): 8 half-tile DMAs over 3
            # queues, issued before anything else, so bn_stats can start on
            # the first 512-col half while later halves are still in flight
            xb_sb = big.tile([128, CT, N], BF16)
            for ci, h in ((0, 0), (0, 1), (2, 0), (2, 1)):
                nc.sync.dma_start(out=xb_sb[:, ci, 512 * h:512 * (h + 1)],
                                  in_=xbf_r[:, ci, 512 * h:512 * (h + 1)])
            for h in range(2):
                nc.scalar.dma_start(out=xb_sb[:, 1, 512 * h:512 * (h + 1)],
                                    in_=xbf_r[:, 1, 512 * h:512 * (h + 1)])

            # ---- PE warm-up: ~10 dummy matmuls on zeros keep the HAM clock
            # gate from throttling the first real matmul stream (PE would
            # otherwise sit idle >3.4us during the x DMA + GN stats). ----
            nc.gpsimd.memset(warm, 0.0)
            for h in range(2):
                nc.gpsimd.dma_start(out=xb_sb[:, 3, 512 * h:512 * (h + 1)],
                                    in_=xbf_r[:, 3, 512 * h:512 * (h + 1)])

            def warm_mm(n):
                for i in range(n):
                    nc.tensor.matmul((psVa if i % 2 == 0 else psVb)[:, 0:512],
                                     warm[:, 0:128], warm[:, 0:512],
                                     start=True, stop=True)

            warm_mm(12)

            # ---- weights (gpsimd queue) in consumption order ----
            # blk col ranges in qkv_blk -> original qkv_wT col ranges
            BLK_MAP = [(0, 0), (128, 512), (256, 1024, 512), (768, 128),
                       (896, 640), (1024, 256), (1152, 768), (1280, 384),
                       (1408, 896)]

            def qkv_load(idx):
                e = BLK_MAP[idx]
                w = e[2] if len(e) == 3 else 128
                nc.gpsimd.dma_start(out=qkv_wT[:, :, e[1]:e[1] + w],
                                    in_=qkv_r[:, :, e[0]:e[0] + w])

            qkv_load(0)                      # q0
            nc.scalar.dma_start(out=sel, in_=sel_d)
            nc.scalar.dma_start(out=expander, in_=expander_d)
            nc.scalar.dma_start(out=gn_w, in_=gn_w_d.rearrange("(t p) o -> p t o", p=128))
            nc.scalar.dma_start(out=gn_b, in_=gn_b_d.rearrange("(t p) o -> p t o", p=128))
            if with_bias:
                nc.scalar.dma_start(out=qk_bias,
                                    in_=qk_bias_d.rearrange("(t p) o -> p t o", p=128))
            qkv_load(1)                      # k0
            qkv_load(2)                      # v (all 512 cols)
            for idx in range(3, 9):          # q1,k1,q2,k2,q3,k3
                qkv_load(idx)
            nc.gpsimd.dma_start(out=proj_wT, in_=proj_r)
            nc.vector.memset(eps_t, EPS)
            nc.vector.memset(ones_b, 1.0)
            # dummy sqrt: forces the Sqrt/Exp ACT table set to load NOW
            # (ACT idle) instead of mid GN chain
            tbl_poke = const.tile([8, 1], F32)
            nc.scalar.activation(out=tbl_poke, in_=eps_t, func=AF.Sqrt)

            # f32 x for the residual, loaded behind the weights (gpsimd)
            x_sb = big.tile([128, CT, N], F32)
            nc.gpsimd.dma_start(out=x_sb, in_=x_r)

            # ---- GroupNorm: cts 0-2 stats on DVE, ct3 on ACT (accum_out =
            # free-dim sums of x and x^2); single sel/expander matmuls ----
            hn = big.tile([128, CT, N], BF16)
            out_sb = big.tile([128, CT, N], F32)
            mv_all = norm.tile([128, CT, 2], F32, bufs=1)
            sums3 = norm.tile([128, 2], F32, bufs=1)
            for ci in range(3):
                bstats = norm.tile([128, 2, 6], F32, tag="bst")
                xv = xb_sb[:, ci, :].rearrange("p (s n) -> p s n", s=2)
                for s in range(2):
                    nc.vector.bn_stats(out=bstats[:, s, :], in_=xv[:, s, :])
                nc.vector.bn_aggr(out=mv_all[:, ci, :], in_=bstats)
            # ct3 on ACT; scratch outputs land in tiles rewritten later
            nc.scalar.activation(out=out_sb[:, 0, :], in_=xb_sb[:, 3, :],
                                 func=AF.Copy, accum_out=sums3[:, 0:1])
            nc.scalar.activation(out=hn[:, 3, :], in_=xb_sb[:, 3, :],
                                 func=AF.Square, accum_out=sums3[:, 1:2])
            # srhs: col0 = mean_c, col1 = E[x^2]
            srhs = norm.tile([128, CT, 2], F32R, bufs=1)
            nc.vector.tensor_copy(out=srhs[:, 0:3, 0], in_=mv_all[:, 0:3, 0])
            nc.vector.tensor_tensor(out=srhs[:, 0:3, 1], in0=mv_all[:, 0:3, 0],
                                    in1=mv_all[:, 0:3, 0], op=ALU.mult)
            nc.vector.tensor_tensor(out=srhs[:, 0:3, 1], in0=srhs[:, 0:3, 1],
                                    in1=mv_all[:, 0:3, 1], op=ALU.add)
            nc.vector.tensor_scalar(out=srhs[:, 3, :], in0=sums3,
                                    scalar1=1.0 / N, scalar2=None, op0=ALU.mult)
            gp = psSc[0:8, 512:512 + 2 * CT]
            nc.tensor.matmul(gp, sel[:], srhs[:], start=True, stop=True)
            warm_mm(4)
            gms = norm.tile([8, CT, 2], F32, bufs=1)
            nc.vector.tensor_copy(out=gms,
                                  in_=gp.rearrange("p (c s) -> p c s", s=2))
            gvar = norm.tile([8, CT], F32, bufs=1)
            grp2 = norm.tile([8, CT, 2], F32R, bufs=1)
            nc.vector.tensor_tensor(out=gvar, in0=gms[:, :, 0], in1=gms[:, :, 0],
                                    op=ALU.mult)
            nc.vector.tensor_tensor(out=gvar, in0=gms[:, :, 1], in1=gvar,
                                    op=ALU.subtract)
            nc.scalar.activation(out=gvar, in_=gvar, func=AF.Sqrt, bias=eps_t,
                                 scale=1.0)
            nc.vector.reciprocal(out=gvar, in_=gvar)
            nc.vector.tensor_copy(out=grp2[:, :, 0:1], in_=gms[:, :, 0:1])
            nc.vector.tensor_copy(out=grp2[:, :, 1],
                                  in_=gvar)
            ep = psSc[:, 576:576 + 2 * CT]
            nc.tensor.matmul(ep, expander[:], grp2[:], start=True, stop=True)
            warm_mm(3)
            eab = norm.tile([128, CT, 2], F32, bufs=1)
            nc.vector.tensor_copy(out=eab,
                                  in_=ep.rearrange("p (c s) -> p c s", s=2))
            A_all = norm.tile([128, CT], F32, bufs=1)
            B_all = norm.tile([128, CT], F32, bufs=1)
            nc.vector.tensor_tensor(out=A_all, in0=eab[:, :, 1],
                                    in1=gn_w[:, :, 0], op=ALU.mult)
            nc.vector.tensor_tensor(out=B_all, in0=eab[:, :, 0], in1=A_all,
                                    op=ALU.mult)
            nc.vector.tensor_tensor(out=B_all, in0=gn_b[:, :, 0], in1=B_all,
                                    op=ALU.subtract)
            # affine split by n-halves so the qk chains (nt0 first) unblock
            # after 4 half-ops instead of 4 full ones
            for ntv in range(NT):
                for ci in range(CT):
                    nc.vector.tensor_scalar(
                        out=hn[:, ci, 512 * ntv:512 * (ntv + 1)],
                        in0=xb_sb[:, ci, 512 * ntv:512 * (ntv + 1)],
                        scalar1=A_all[:, ci:ci + 1],
                        scalar2=B_all[:, ci:ci + 1],
                        op0=ALU.mult, op1=ALU.add)

            # ---- data tiles for attention ----
            q_sb = big.tile([128, CT, N], BF16)
            k_sb = big.tile([128, CT, N], BF16)
            vT = big.tile([128, MT, NH, D + 2], BF16)
            oT = big.tile([128, CT, N], BF16)
            nc.vector.memset(vT[:, :, :, D:D + 1], 1.0)
            nc.vector.memset(vT[:, :, :, D + 1:D + 2], 0.0)

            # ---- emission helpers ----
            def qk_tile(i, on_act, bases):
                """QKV tile i (0-3 = q ct, 4-7 = k ct). nt-outer: the nt1
                chain's first MM is FIFO-gated behind nt0's kt3 (needs hn3),
                so psSc's GN half is only written once GN is done."""
                dest = q_sb if i < CT else k_sb
                ci = i % CT
                base = bases[0]
                for nt in range(NT):
                    for kt in range(CT):
                        nc.tensor.matmul(
                            base[:, 512 * nt:512 * (nt + 1)],
                            qkv_wT[:, kt, 128 * i:128 * (i + 1)],
                            hn[:, kt, 512 * nt:512 * (nt + 1)],
                            start=(kt == 0), stop=(kt == CT - 1))
                if with_bias:
                    nc.vector.tensor_scalar(out=dest[:, ci, :],
                                            in0=base[:, 0:1024],
                                            scalar1=qk_bias[:, i, :],
                                            scalar2=None, op0=ALU.add)
                elif on_act:
                    nc.scalar.activation(out=dest[:, ci, :],
                                         in_=base[:, 0:1024], func=AF.Copy)
                else:
                    nc.vector.tensor_copy(out=dest[:, ci, :],
                                          in_=base[:, 0:1024])

            def v_tile(mt):
                """v for n-block mt, evac to vT (head-interleaved)."""
                base = psVa if mt % 2 == 0 else psVb
                for kt in range(CT):
                    nc.tensor.matmul(base[:, 0:512],
                                     hn[:, kt, 128 * mt:128 * (mt + 1)],
                                     qkv_wT[:, kt, 2 * C:3 * C],
                                     start=(kt == 0), stop=(kt == CT - 1))
                nc.vector.tensor_copy(
                    out=vT[:, mt, :, 0:D],
                    in_=base[:, 0:512].rearrange("p (h d) -> p h d", h=NH))

            pT_tiles = {}

            def st_half(t, g, on_dve, half):
                """S^T for head pair t, group g = 2*mt + hh, into psS tile
                `half`; exp on ACT or DVE-Schraudolph."""
                mt, hh = g // 2, g % 2
                qp = hh * 64
                for nt in range(NT):
                    nc.tensor.matmul(
                        half[:, 512 * nt:512 * (nt + 1)],
                        k_sb[qp:qp + 64, t, 128 * mt:128 * (mt + 1)],
                        q_sb[qp:qp + 64, t, 512 * nt:512 * (nt + 1)],
                        start=True, stop=True)
                pTt = pT_tiles[t]
                if on_dve:
                    nc.vector.tensor_scalar(
                        out=pTt.bitcast(I16)[:, hh, mt, :], in0=half,
                        scalar1=SCH_A, scalar2=SCH_B,
                        op0=ALU.mult, op1=ALU.add)
                else:
                    nc.scalar.activation(out=pTt[:, hh, mt, :], in_=half,
                                         func=AF.Exp, scale=scale)

            # Chains 2,3 of each pair close EARLY (position 2/6) and use the
            # DRAM-bounce reciprocal broadcast (long latency, no PE cost).
            # Chains 0,1 close LATE (position 11/15) and use the low-latency
            # self-tile PE broadcast (rows 64:128 of their own psV tile).
            # Pair 3 (the tail) is all self-tile.
            def pv_tile(t, chain):
                if t == 3:
                    # hh-major: chains 0,1 (nt0) in psVa; 2,3 (nt1) in psVb
                    return psVa if chain in (0, 1) else psVb
                return psVa if chain % 2 == 0 else psVb

            def pv_chunk(t, chain, part):
                """4 MMs of PV chain (0=h_ev/nt0, 1=h_od/nt0, 2=h_ev/nt1,
                3=h_od/nt1), part 0/1 = m-tiles 0-3 / 4-7."""
                hh = chain % 2
                nt = chain // 2
                h = 2 * t + hh
                tile_ = pv_tile(t, chain)
                slot = tile_[0:D + 2, 0:512]
                pTt = pT_tiles[t]
                for mt in range(4 * part, 4 * part + 4):
                    nc.tensor.matmul(slot,
                                     vT[:, mt, h, :],
                                     pTt[:, hh, mt, 512 * nt:512 * (nt + 1)],
                                     start=(mt == 0), stop=(mt == MT - 1))
                if part == 1:
                    if t < 3 and chain in (2, 3):
                        # denominator row -> SBUF, approx reciprocal, then
                        # DRAM-bounce broadcast of the reciprocal to 64 rows
                        rs = small.tile([1, 512], F32, tag="rs", bufs=4,
                                        name=f"rs_{h}_{nt}")
                        rs2 = small.tile([1, 512], F32, tag="rs2", bufs=4,
                                         name=f"rs2_{h}_{nt}")
                        nc.vector.tensor_copy(out=rs,
                                              in_=tile_[D:D + 1, 0:512])
                        nc.vector.reciprocal_approx_fast(out=rs2, in_=rs)
                        nc.sync.dma_start(
                            out=rs2_dram[h:h + 1, 512 * nt:512 * (nt + 1)],
                            in_=rs2)
                        bc_load(t, chain)
                    else:
                        denom_self(t, chain)

            bc_tiles = {}

            def bc_load(t, chain):
                """Broadcast 1/denom to 64 partitions via DRAM-bounce DMA."""
                hh, nt = chain % 2, chain // 2
                h = 2 * t + hh
                qp = hh * 64
                key = (t, nt)
                if key not in bc_tiles:
                    bc_tiles[key] = small.tile([128, 512], F32, tag=f"bc{nt}",
                                               bufs=2, name=f"bc_{t}_{nt}")
                bc = bc_tiles[key]
                srcap = rs2_dram[h:h + 1, 512 * nt:512 * (nt + 1)]
                nc.gpsimd.dma_start(out=bc[qp:qp + 64, :],
                                    in_=bass.AP(tensor=srcap.tensor,
                                                offset=srcap.offset,
                                                ap=[[0, 64]] + list(srcap.ap[1:])))

            def pv_evac(t, chain):
                """Fused evacuate+normalize: oT = psum_o * (1/denom)."""
                hh, nt = chain % 2, chain // 2
                qp = hh * 64
                tile_ = pv_tile(t, chain)
                bc = bc_tiles[(t, nt)]
                nc.vector.tensor_tensor(
                    out=oT[qp:qp + 64, t, 512 * nt:512 * (nt + 1)],
                    in0=tile_[0:D, 0:512], in1=bc[qp:qp + 64, :], op=ALU.mult)

            def denom_self(t, chain):
                """Recip of the psum denominator row, bf16-cast on ACT, then
                PE-broadcast into the SAME psV tile's rows 64:128 (no
                cross-tile lattice, no DRAM bounce latency)."""
                tile_ = pv_tile(t, chain)
                rs = small.tile([1, 512], F32, tag="rs", bufs=4,
                                name=f"rst_{t}_{chain}")
                rs2 = small.tile([1, 512], F32, tag="rs2", bufs=4,
                                 name=f"rs2t_{t}_{chain}")
                rs2b = small.tile([1, 512], BF16, tag="rs2b", bufs=4,
                                  name=f"rs2b_{t}_{chain}")
                # rs -> recip -> bf16 cast all back-to-back on DVE: no ACT
                # queue hop on the chain's critical path (ACT tail queue has
                # head-of-line blocking with 1-7us waits)
                nc.vector.tensor_copy(out=rs, in_=tile_[D:D + 1, 0:512])
                nc.vector.reciprocal_approx_fast(out=rs2, in_=rs)
                nc.vector.tensor_copy(out=rs2b, in_=rs2)
                nc.tensor.matmul(tile_[64:128, 0:512], ones_b, rs2b,
                                 start=True, stop=True)

            def pv_evac_self(t, chain):
                hh, nt = chain % 2, chain // 2
                qp = hh * 64
                tile_ = pv_tile(t, chain)
                # stage the broadcast rows to SBUF (ACT when idle in the
                # tail, DVE mid-kernel where ACT is exp-bound), multiply on
                # DVE.  (Both-operands-in-PSUM TT crashes the backend.)
                bcs = small.tile([64, 512], F32, tag="bcs", bufs=2,
                                 name=f"bcs_{t}_{chain}")
                if t == 3:
                    nc.scalar.activation(out=bcs, in_=tile_[64:128, 0:512],
                                         func=AF.Copy)
                else:
                    nc.vector.tensor_copy(out=bcs, in_=tile_[64:128, 0:512])
                nc.vector.tensor_tensor(
                    out=oT[qp:qp + 64, t, 512 * nt:512 * (nt + 1)],
                    in0=tile_[0:D, 0:512], in1=bcs, op=ALU.mult)

            def proj_part(ot, nt, kts, base, bc0, first=False, last=False):
                for kt in kts:
                    nc.tensor.matmul(base[:, bc0:bc0 + 512],
                                     proj_wT[:, kt, 128 * ot:128 * (ot + 1)],
                                     oT[:, kt, 512 * nt:512 * (nt + 1)],
                                     start=(first and kt == kts[0]),
                                     stop=(last and kt == kts[-1]))

            def proj_out(ot, nt, base, bc0, eng):
                """Evacuate one proj half + residual (proj bias folded into
                x on host) and DMA it out. eng 0 = DVE, 1 = ACT + gpsimd."""
                ocols = slice(512 * nt, 512 * (nt + 1))
                if eng == 0:
                    nc.vector.tensor_tensor(out=out_sb[:, ot, ocols],
                                            in0=base[:, bc0:bc0 + 512],
                                            in1=x_sb[:, ot, ocols], op=ALU.add)
                else:
                    nc.scalar.activation(out=out_sb[:, ot, ocols],
                                         in_=base[:, bc0:bc0 + 512],
                                         func=AF.Copy)
                    nc.gpsimd.tensor_tensor(out=out_sb[:, ot, ocols],
                                            in0=out_sb[:, ot, ocols],
                                            in1=x_sb[:, ot, ocols],
                                            op=ALU.add)
                q = nc.sync if (ot + nt) % 2 == 0 else nc.scalar
                q.dma_start(out=out_r[:, ot, 512 * nt:512 * (nt + 1)],
                            in_=out_sb[:, ot, ocols])

            def alloc_pT(t):
                pT_tiles[t] = pT_pool.tile([128, 2, MT, N], BF16, tag="pT", bufs=2,
                                           name=f"pT_{t}")

            # ---- pipeline emission ----
            # q0, k0 first (psSa/psSb) so pair-0 S^T can start early
            qk_tile(0, on_act=True, bases=[psSa])
            qk_tile(4, on_act=True, bases=[psSb])

            # pair 0: S^T+exp (all ACT; PE-bound anyway) + v tiles (psV) +
            # remaining qk tiles (psX)
            alloc_pT(0)
            rest = [1, 5, 2, 6, 3, 7]        # q1,k1,q2,k2,q3,k3
            p0rot = [psSa, psSb]
            for g in range(16):
                if g % 2 == 0:
                    v_tile(g // 2)
                elif g < 13:
                    # q1/k1 evacs ride ACT (DVE is pair-0's busiest engine)
                    qk_tile(rest[g // 2], on_act=(g < 5), bases=[psSc])
                st_half(0, g, on_dve=False, half=p0rot[g % 2])

            # pairs 1..3: PV(t-1) + S^T(t) + exp (ACT/DVE split) + stage_b.
            # pair 3 emits S^T hh-major so PV(3)'s hh0 chains unblock before
            # the last exps land.
            for t in range(1, 4):
                alloc_pT(t)
                pv = t - 1
                chunk_sched = {0: (2, 0), 2: (2, 1), 4: (3, 0), 6: (3, 1),
                               9: (0, 0), 11: (0, 1), 13: (1, 0), 15: (1, 1)}
                order = ([0, 2, 4, 6, 8, 10, 12, 14, 1, 3, 5, 7, 9, 11, 13, 15]
                         if t == 3 else list(range(16)))
                rot = [psSa, psSb, psSc]
                for idx in range(16):
                    if idx == 1 and pv >= 1:
                        pv_evac_self(pv - 1, 1)
                    cs = chunk_sched.get(idx)
                    if cs is not None:
                        pv_chunk(pv, cs[0], cs[1])
                    st_half(t, order[idx], on_dve=(idx in EXP_DVE),
                            half=rot[idx % 3])
                    if idx == 8:
                        pv_evac(pv, 2)
                    elif idx == 12:
                        pv_evac(pv, 3)
                    elif idx == 13:
                        pv_evac_self(pv, 0)
                del pT_tiles[t - 1]

            # ---- tail: PV(3) hh0 chains first, per-(ot,nt) proj chains with
            # early half-tile evac + DMA; denominators via self-tile PE
            # broadcast (rows 64:128 of the chain's own psV tile). ----
            pv_evac_self(2, 1)                    # frees psVb
            pv_chunk(3, 0, 0)
            pv_chunk(3, 0, 1)                     # psVa chain0 (hh0 nt0)
            pv_chunk(3, 2, 0)
            pv_chunk(3, 2, 1)                     # psVb chain2 (hh0 nt1)
            pv_evac_self(3, 0)                    # oT[0:64, 3, 0:512]
            proj_part(0, 0, [0, 1, 2], psSb, 0, first=True)
            pv_evac_self(3, 2)                    # oT[0:64, 3, 512:1024]
            pv_chunk(3, 1, 0)
            pv_chunk(3, 1, 1)                     # psVa chain1 (hh1 nt0)
            proj_part(0, 1, [0, 1, 2], psSb, 512, first=True)
            pv_evac_self(3, 1)                    # oT[64:128, 3, 0:512]
            proj_part(1, 0, [0, 1, 2], psSc, 0, first=True)
            pv_chunk(3, 3, 0)
            pv_chunk(3, 3, 1)                     # psVb chain3 (hh1 nt1)
            proj_part(0, 0, [3], psSb, 0, last=True)
            proj_out(0, 0, psSb, 0, eng=1)
            proj_part(1, 1, [0, 1, 2], psSc, 512, first=True)
            pv_evac_self(3, 3)                    # oT[64:128, 3, 512:1024]
            proj_part(2, 0, [0, 1, 2], psSa, 0, first=True)
            proj_part(1, 0, [3], psSc, 0, last=True)
            proj_out(1, 0, psSc, 0, eng=0)
            proj_part(3, 0, [0, 1, 2, 3], psVa, 0, first=True, last=True)
            proj_part(2, 1, [0, 1, 2], psSa, 512, first=True)
            proj_out(3, 0, psVa, 0, eng=1)
            proj_part(2, 0, [3], psSa, 0, last=True)
            proj_out(2, 0, psSa, 0, eng=0)
            proj_part(0, 1, [3], psSb, 512, last=True)
            proj_out(0, 1, psSb, 512, eng=1)
            proj_part(3, 1, [0, 1, 2, 3], psVb, 0, first=True, last=True)
            proj_part(1, 1, [3], psSc, 512, last=True)
            proj_out(1, 1, psSc, 512, eng=0)
            proj_out(3, 1, psVb, 0, eng=0)
            proj_part(2, 1, [3], psSa, 512, last=True)
            proj_out(2, 1, psSa, 512, eng=0)

    nc.compile()
    return nc


def _host_prep(x, gn_w, gn_b, qkv_w, qkv_b, proj_w, proj_b):
    xf = np.ascontiguousarray(x.reshape(B, C, N), dtype=np.float32)
    import ml_dtypes
    qkv_wT = np.ascontiguousarray(qkv_w.T).astype(ml_dtypes.bfloat16)
    # permute columns into consumption order (see BLK_MAP in _build)
    qkv_blk = np.ascontiguousarray(np.concatenate(
        [qkv_wT[:, 0:128], qkv_wT[:, 512:640], qkv_wT[:, 1024:1536],
         qkv_wT[:, 128:256], qkv_wT[:, 640:768], qkv_wT[:, 256:384],
         qkv_wT[:, 768:896], qkv_wT[:, 384:512], qkv_wT[:, 896:1024]],
        axis=1))
    proj_wT = np.ascontiguousarray(proj_w.T).astype(ml_dtypes.bfloat16)
    # fold the proj bias (incl. v-bias pushed through proj_w) into the
    # residual copy of x; the bf16 GN input stays un-folded
    proj_be = (proj_b + proj_w @ qkv_b[2 * C:]).astype(np.float32)
    qk_bias = np.ascontiguousarray(qkv_b[:2 * C], dtype=np.float32).reshape(2 * C, 1)
    cid = np.arange(128)
    sel = ((cid[:, None] // GS == np.arange(8)[None, :]) / GS).astype(np.float32)
    expander = np.ascontiguousarray(
        (cid[:, None] // GS == np.arange(8)[None, :]).T.astype(np.float32))
    shared = {
        "qkv_blk": qkv_blk, "proj_wT": proj_wT, "qk_bias": qk_bias,
        "gn_w": np.asarray(gn_w, np.float32).reshape(C, 1),
        "gn_b": np.asarray(gn_b, np.float32).reshape(C, 1),
        "sel": sel, "expander": expander,
    }
    return [{**shared,
             "x": np.ascontiguousarray(xf[i] + proj_be[:, None]),
             "x_bf": xf[i].astype(ml_dtypes.bfloat16)} for i in range(B)]


def kernel(x, gn_w, gn_b, qkv_w, qkv_b, proj_w, proj_b):
    from concourse import bass_utils
    in_maps = _host_prep(np.asarray(x), np.asarray(gn_w), np.asarray(gn_b),
                         np.asarray(qkv_w), np.asarray(qkv_b),
                         np.asarray(proj_w), np.asarray(proj_b))
    with_bias = bool(np.any(np.asarray(qkv_b)[:2 * C] != 0.0))
    key = ("nc", with_bias)
    if key not in _cache:
        _cache[key] = _build(with_bias)
    res = bass_utils.run_bass_kernel_spmd(_cache[key], in_maps,
                                          core_ids=list(range(B)), trace=TRACE)
    _cache["last_result"] = res
    out = np.stack([res.results[i]["out"] for i in range(B)])
    return out.reshape(B, C, 32, 32).astype(np.float32)

